# revision 1
# baseline (speedup 1.0000x reference)
"""Mixture-of-Experts (E=8, top-2) — expert-parallel Trainium2 Bass kernel.

Strategy (expert-parallel, per the sharding hint):
  * Host computes the router (logits -> top-2 -> softmax) in numpy; this is the
    token->core sharding decision.
  * Tokens are gathered per expert, padded to a fixed capacity C, and core e
    runs expert e's full MLP  y = w2 @ gelu(w1 @ x + b1) + b2  over its tokens
    (bf16 matmuls, fp32 PSUM accumulation, exact-gelu ACT eviction).
  * Host applies the top-2 combine weights and scatter-adds the two expert
    outputs per token (the unshard step).

Device layout (per core, SPMD — same program, per-core data):
  xt  [D, C]  bf16   gathered tokens, transposed (token dim = free dim)
  w1t [D, F]  bf16   w1[e].T     (contraction dim D on partitions)
  w2t [F, D]  bf16   w2[e].T     (contraction dim F on partitions)
  b1r [128, F/128]   b1 as per-partition bias table
  b2r [128, D/128]   b2 as per-partition bias table
  yt  [D, C]  f32    expert output, transposed

Both weight matrices stay resident in SBUF; tokens stream in chunks of NT.
"""

import numpy as np
from contextlib import ExitStack

from ml_dtypes import bfloat16

import concourse.bacc as bacc
import concourse.tile as tile
import concourse.mybir as mybir
from concourse.bass_utils import run_bass_kernel_spmd

P = 128
D = 1024
F = 4096
E = 8
TOPK = 2
B, S = 4, 2048

NT = 272           # tokens per inner chunk (matmul free dim)
C_DEFAULT = 2176   # padded per-expert token capacity (multiple of NT)

KD = D // P        # 8  k-tiles for MLP1 (contract D)
KF = F // P        # 32 k-tiles for MLP2 (contract F)
MF = F // P        # 32 m-tiles for MLP1 output (F)
MD = D // P        # 8  m-tiles for MLP2 output (D)

_prog_cache: dict = {}
ACT_FUNC = None  # default: Gelu; sim_check overrides (CoreSim lacks Gelu)
last_results = None  # BassKernelResults of the most recent run (for test harness)
trace_kwargs: dict = {}  # test harness can set e.g. {"trace": True}


def _build_program(C: int):
    """Build + compile the SPMD single-expert MLP program for capacity C."""
    bf16 = mybir.dt.bfloat16
    f32 = mybir.dt.float32
    nchunks = C // NT

    nc = bacc.Bacc(
        "TRN2",
        target_bir_lowering=False,
        debug=False,
        enable_asserts=False,
        num_devices=E,
    )

    xt = nc.dram_tensor("xt", [D, C], bf16, kind="ExternalInput").ap()
    w1t = nc.dram_tensor("w1t", [D, F], bf16, kind="ExternalInput").ap()
    w2t = nc.dram_tensor("w2t", [F, D], bf16, kind="ExternalInput").ap()
    b1r = nc.dram_tensor("b1r", [P, MF], f32, kind="ExternalInput").ap()
    b2r = nc.dram_tensor("b2r", [P, MD], f32, kind="ExternalInput").ap()
    yt = nc.dram_tensor("yt", [D, C], f32, kind="ExternalOutput").ap()

    # Partition-tiled DRAM views: one 3D-AP DMA replaces 8-32 row-block DMAs
    # (DMA issue costs ~650ns of engine time each — batching matters).
    xt_r = xt.rearrange("(k p) c -> p k c", p=P)  # [128, KD, C]
    w1t_r = w1t.rearrange("(k p) f -> p k f", p=P)  # [128, KD, F]
    w2t_r = w2t.rearrange("(k p) d -> p k d", p=P)  # [128, KF, D]
    yt_r = yt.rearrange("(m p) c -> p m c", p=P)  # [128, MD, C]

    with tile.TileContext(nc) as tc, ExitStack() as ctx:
        wpool = ctx.enter_context(tc.tile_pool(name="wpool", bufs=1))
        xpool = ctx.enter_context(tc.tile_pool(name="xpool", bufs=2))
        hpool = ctx.enter_context(tc.tile_pool(name="hpool", bufs=2))
        ypool = ctx.enter_context(tc.tile_pool(name="ypool", bufs=2))
        pspool = ctx.enter_context(tc.tile_pool(name="pspool", bufs=4, space="PSUM"))

        def load_x(ci):
            t = xpool.tile([P, KD, NT], bf16, name="xtile")
            nc.sync.dma_start(out=t[:, :, :], in_=xt_r[:, :, ci * NT : (ci + 1) * NT])
            return t

        # Chunk 0's tokens first so PE can start as soon as w1's first quarter lands.
        x_first = load_x(0)

        # Weight streams are consumed strictly in order, so keep each stream on
        # ONE queue (queues share the ~360GB/s HBM port; interleaving an
        # ordered stream across queues of unequal rate stalls the consumer).
        # gpsimd (SWDGE) carries w1 pieces then w2 groups 1-3; sync carries
        # x chunks, biases, and w2 group 0 (needed right as chunk-0 MLP2
        # starts, before gpsimd finishes w1).
        b1_sb = wpool.tile([P, MF], f32, name="b1sb")
        nc.sync.dma_start(out=b1_sb[:, :], in_=b1r[:, :])
        b2_sb = wpool.tile([P, MD], f32, name="b2sb")
        nc.sync.dma_start(out=b2_sb[:, :], in_=b2r[:, :])
        NQ = 8
        FQ = F // NQ  # 512
        w1_sb = []
        for q in range(NQ):
            t = wpool.tile([P, KD, FQ], bf16, name=f"w1_{q}")
            nc.gpsimd.dma_start(out=t[:, :, :], in_=w1t_r[:, :, q * FQ : (q + 1) * FQ])
            w1_sb.append(t)
        KG = 4
        KGS = KF // KG  # 8
        w2_sb = []
        for g in range(KG):
            t = wpool.tile([P, KGS, D], bf16, name=f"w2_{g}")
            eng = nc.sync if g == 0 else nc.gpsimd
            eng.dma_start(out=t[:, :, :], in_=w2t_r[:, g * KGS : (g + 1) * KGS, :])
            w2_sb.append(t)

        for ci in range(nchunks):
            x_sb = x_first if ci == 0 else load_x(ci)

            # MLP1: h[F, NT] = gelu(w1t.T @ x + b1), bf16 out
            h_sb = hpool.tile([P, KF, NT], bf16, name="htile")
            for m in range(MF):
                q, mq = divmod(m, MF // NQ)
                pt = pspool.tile([P, NT], f32, name="pt")
                for k in range(KD):
                    nc.tensor.matmul(
                        pt[:, :],
                        lhsT=w1_sb[q][:, k, mq * P : (mq + 1) * P],
                        rhs=x_sb[:, k, :],
                        start=(k == 0),
                        stop=(k == KD - 1),
                    )
                nc.scalar.activation(
                    h_sb[:, m, :],
                    pt[:, :],
                    ACT_FUNC or mybir.ActivationFunctionType.Gelu,
                    bias=b1_sb[:, m : m + 1],
                )

            # MLP2: y[D, NT] = w2t.T @ h + b2, f32 out
            y_sb = ypool.tile([P, MD, NT], f32, name="ytile")
            for m in range(MD):
                pt = pspool.tile([P, NT], f32, name="pt")
                for k in range(KF):
                    g, kg = divmod(k, KGS)
                    nc.tensor.matmul(
                        pt[:, :],
                        lhsT=w2_sb[g][:, kg, m * P : (m + 1) * P],
                        rhs=h_sb[:, k, :],
                        start=(k == 0),
                        stop=(k == KF - 1),
                    )
                nc.scalar.activation(
                    y_sb[:, m, :],
                    pt[:, :],
                    mybir.ActivationFunctionType.Identity,
                    bias=b2_sb[:, m : m + 1],
                )
                if m == MD // 2 - 1 or m == MD - 1:
                    h0 = m + 1 - MD // 2
                    nc.scalar.dma_start(
                        out=yt_r[:, h0 : m + 1, ci * NT : (ci + 1) * NT],
                        in_=y_sb[:, h0 : m + 1, :],
                    )

    nc.compile()
    return nc


def _get_program(C: int):
    if C not in _prog_cache:
        _prog_cache[C] = _build_program(C)
    return _prog_cache[C]


def _route(xf: np.ndarray, router_w: np.ndarray):
    """Top-2 routing identical to the reference (ties -> lower expert idx).

    Logits in fp64 so the selection is independent of BLAS blocking/threads
    (top-2 gaps in this regime are >= ~3e-6; fp64 noise is ~1e-15).
    """
    logits = xf.astype(np.float64) @ router_w.T.astype(np.float64)  # [T, E]
    idx = np.argsort(-logits, axis=1, kind="stable")[:, :TOPK]
    vals = np.take_along_axis(logits, idx, axis=1)
    vals = vals - vals.max(axis=1, keepdims=True)
    ev = np.exp(vals)
    probs = (ev / ev.sum(axis=1, keepdims=True)).astype(np.float32)
    return idx.astype(np.int64), probs


def kernel(x, router_w, w1, b1, w2, b2):
    global last_results

    x = np.asarray(x, dtype=np.float32)
    router_w = np.asarray(router_w, dtype=np.float32)
    w1 = np.asarray(w1, dtype=np.float32)
    b1 = np.asarray(b1, dtype=np.float32)
    w2 = np.asarray(w2, dtype=np.float32)
    b2 = np.asarray(b2, dtype=np.float32)

    orig_shape = x.shape
    xf = x.reshape(-1, D)
    T = xf.shape[0]

    idx, probs = _route(xf, router_w)

    # Group the (token, k) pairs by expert; rank = position within the expert.
    flat_e = idx.ravel()  # entry j corresponds to token j//2, slot j%2
    order = np.argsort(flat_e, kind="stable")
    counts = np.bincount(flat_e, minlength=E)
    starts = np.zeros(E + 1, dtype=np.int64)
    np.cumsum(counts, out=starts[1:])
    rank = np.empty(2 * T, dtype=np.int64)
    rank[order] = np.arange(2 * T, dtype=np.int64) - starts[flat_e[order]]
    pos = rank.reshape(T, TOPK)

    cmax = int(counts.max())
    C = C_DEFAULT if cmax <= C_DEFAULT else int(-(-cmax // NT) * NT)
    nc = _get_program(C)

    xf_bf = xf.astype(bfloat16)
    in_maps = []
    for e in range(E):
        toks = order[starts[e] : starts[e + 1]] // 2
        xt = np.zeros((D, C), dtype=bfloat16)
        xt[:, : len(toks)] = xf_bf[toks].T
        in_maps.append(
            {
                "xt": xt,
                "w1t": np.ascontiguousarray(w1[e].T).astype(bfloat16),
                "w2t": np.ascontiguousarray(w2[e].T).astype(bfloat16),
                "b1r": np.ascontiguousarray(b1[e].reshape(MF, P).T),
                "b2r": np.ascontiguousarray(b2[e].reshape(MD, P).T),
            }
        )

    res = run_bass_kernel_spmd(nc, in_maps, core_ids=list(range(E)), **trace_kwargs)
    last_results = res

    ys = np.stack([np.asarray(r["yt"]) for r in res.results])  # [E, D, C]
    out = probs[:, 0:1] * ys[idx[:, 0], :, pos[:, 0]]
    out += probs[:, 1:2] * ys[idx[:, 1], :, pos[:, 1]]
    return out.astype(np.float32).reshape(orig_shape)



# revision 3
# speedup vs baseline: 1.0304x; 1.0304x over previous
"""Mixture-of-Experts (E=8, top-2) — F-sliced Trainium2 Bass kernel.

Strategy (intermediate-dim sharding; perfectly load-balanced):
  * Host computes the router (logits -> top-2 -> softmax) in numpy and sorts
    the 2*T (token, slot) pairs by expert.
  * Core c keeps ALL 8 experts' weights resident, but only the F-column slice
    [512c, 512(c+1)) of each — 16.8 MB of bf16, fits SBUF.  Every core streams
    ALL pairs through its slice:  y_part = w2[e][:, fs].T' @ gelu(w1[e][fs] @ x
    + b1[fs]).  Partials are evicted in bf16 and summed on the host (+ b2 and
    the top-2 prob combine).
  * Because every core runs every pair, the work is identical on all cores no
    matter how tokens route: 16384 matmul columns each, zero capacity padding.
    The chunk schedule (chunks never straddle an expert boundary) is baked
    into the program from the exact per-expert counts.

Device layout (per core, SPMD — same program, per-core weight slices):
  xt   [D, TP]      bf16  all pairs, expert-sorted, token dim = free dim
  w1s  [E, D, FS]   bf16  w1[e].T column-slice   (contract D on partitions)
  w2s  [E, FS, D]   bf16  w2[e].T row-slice      (contract FS on partitions)
  b1s  [P, E, MF]   f32   b1 slice as per-partition bias table
  yt   [D, TP]      bf16  partial expert outputs (summed across cores on host)
"""

import numpy as np
from contextlib import ExitStack

from ml_dtypes import bfloat16

import concourse.bacc as bacc
import concourse.tile as tile
import concourse.mybir as mybir
from concourse.bass_utils import run_bass_kernel_spmd

P = 128
D = 1024
F = 4096
E = 8
TOPK = 2
B, S = 4, 2048
T = B * S
TP = TOPK * T      # 16384 (token, slot) pairs, each a matmul column

FS = F // E        # 512  F-slice width per core
NT = 512           # max tokens per chunk (matmul moving free dim limit)

KD = D // P        # 8  k-tiles for MLP1 (contract D)
MF = FS // P       # 4  m-tiles for MLP1 output (F slice)
KS = FS // P       # 4  k-tiles for MLP2 (contract F slice)
MD = D // P        # 8  m-tiles for MLP2 output (D)

_prog_cache: dict = {}
ACT_FUNC = None  # default: Gelu; sim_check overrides (CoreSim lacks Gelu)
last_results = None  # BassKernelResults of the most recent run (for test harness)
trace_kwargs: dict = {}  # test harness can set e.g. {"trace": True}


def _schedule(counts):
    """Chunk schedule [(expert, col_offset, width), ...] — no chunk straddles
    an expert boundary; widths <= NT; total width == sum(counts)."""
    sched = []
    off = 0
    for e in range(E):
        left = int(counts[e])
        while left > 0:
            n = min(NT, left)
            sched.append((e, off, n))
            off += n
            left -= n
    return tuple(sched)


def _build_program(sched):
    """Build + compile the SPMD F-sliced all-experts program."""
    bf16 = mybir.dt.bfloat16
    f32 = mybir.dt.float32

    nc = bacc.Bacc(
        "TRN2",
        target_bir_lowering=False,
        debug=False,
        enable_asserts=False,
        num_devices=E,
    )

    xt = nc.dram_tensor("xt", [D, TP], bf16, kind="ExternalInput").ap()
    w1s = nc.dram_tensor("w1s", [E, D, FS], bf16, kind="ExternalInput").ap()
    w2s = nc.dram_tensor("w2s", [E, FS, D], bf16, kind="ExternalInput").ap()
    b1s = nc.dram_tensor("b1s", [P, E, MF], f32, kind="ExternalInput").ap()
    yt = nc.dram_tensor("yt", [D, TP], bf16, kind="ExternalOutput").ap()

    # Partition-tiled DRAM views (one multi-dim AP DMA instead of many
    # row-block DMAs; DMA issue costs ~650ns of engine time each).
    xt_r = xt.rearrange("(k p) t -> p k t", p=P)      # [128, KD, TP]
    w1s_r = w1s.rearrange("e (k p) f -> p e k f", p=P)  # [128, E, KD, FS]
    w2s_r = w2s.rearrange("e (k p) d -> p e k d", p=P)  # [128, E, KS, D]
    yt_r = yt.rearrange("(m p) t -> p m t", p=P)      # [128, MD, TP]

    with tile.TileContext(nc) as tc, ExitStack() as ctx:
        wpool = ctx.enter_context(tc.tile_pool(name="wpool", bufs=1))
        xpool = ctx.enter_context(tc.tile_pool(name="xpool", bufs=3))
        hpool = ctx.enter_context(tc.tile_pool(name="hpool", bufs=2))
        ypool = ctx.enter_context(tc.tile_pool(name="ypool", bufs=3))
        ps1 = ctx.enter_context(tc.tile_pool(name="ps1", bufs=3, space="PSUM"))
        ps2 = ctx.enter_context(tc.tile_pool(name="ps2", bufs=3, space="PSUM"))

        # Expert 0's weights are latency-critical (compute reaches expert e's
        # segment ~55us*e in, but expert 0 is needed immediately).  sync
        # carries b1 + w2[0] then the x chunks; gpsimd (SWDGE) carries w1[0]
        # first, then the rest.
        b1_sb = wpool.tile([P, E, MF], f32, name="b1sb")
        nc.sync.dma_start(out=b1_sb[:, :, :], in_=b1s[:, :, :])
        w1_sb = [wpool.tile([P, KD, FS], bf16, name=f"w1_{e}") for e in range(E)]
        w2_sb = [wpool.tile([P, KS, D], bf16, name=f"w2_{e}") for e in range(E)]
        nc.gpsimd.dma_start(out=w1_sb[0][:, :, :], in_=w1s_r[:, 0])
        nc.sync.dma_start(out=w2_sb[0][:, :, :], in_=w2s_r[:, 0])
        for e in range(1, E):
            nc.gpsimd.dma_start(out=w1_sb[e][:, :, :], in_=w1s_r[:, e])
            nc.gpsimd.dma_start(out=w2_sb[e][:, :, :], in_=w2s_r[:, e])

        for e, off, n in sched:
            x_sb = xpool.tile([P, KD, NT], bf16, name="xtile")
            nc.sync.dma_start(out=x_sb[:, :, :n], in_=xt_r[:, :, off : off + n])

            # MLP1: h[FS, n] = gelu(w1s[e].T @ x + b1s[e]), bf16 out
            h_sb = hpool.tile([P, KS, NT], bf16, name="htile")
            for m in range(MF):
                pt = ps1.tile([P, NT], f32, name="p1")
                for k in range(KD):
                    nc.tensor.matmul(
                        pt[:, :n],
                        lhsT=w1_sb[e][:, k, m * P : (m + 1) * P],
                        rhs=x_sb[:, k, :n],
                        start=(k == 0),
                        stop=(k == KD - 1),
                    )
                nc.scalar.activation(
                    h_sb[:, m, :n],
                    pt[:, :n],
                    ACT_FUNC or mybir.ActivationFunctionType.Gelu,
                    bias=b1_sb[:, e, m : m + 1],
                )

            # MLP2 partial: y[D, n] = w2s[e].T @ h, bf16 out (b2 on host)
            y_sb = ypool.tile([P, MD, NT], bf16, name="ytile")
            for m in range(MD):
                pt = ps2.tile([P, NT], f32, name="p2")
                for k in range(KS):
                    nc.tensor.matmul(
                        pt[:, :n],
                        lhsT=w2_sb[e][:, k, m * P : (m + 1) * P],
                        rhs=h_sb[:, k, :n],
                        start=(k == 0),
                        stop=(k == KS - 1),
                    )
                nc.vector.tensor_copy(out=y_sb[:, m, :n], in_=pt[:, :n])
            nc.scalar.dma_start(
                out=yt_r[:, :, off : off + n], in_=y_sb[:, :, :n]
            )

    nc.compile()
    return nc


def _get_program(sched):
    if sched not in _prog_cache:
        _prog_cache[sched] = _build_program(sched)
    return _prog_cache[sched]


def _route(xf: np.ndarray, router_w: np.ndarray):
    """Top-2 routing identical to the reference (ties -> lower expert idx).

    Logits in fp64 so the selection is independent of BLAS blocking/threads
    (top-2 gaps in this regime are >= ~3e-6; fp64 noise is ~1e-15).
    """
    logits = xf.astype(np.float64) @ router_w.T.astype(np.float64)  # [T, E]
    idx = np.argsort(-logits, axis=1, kind="stable")[:, :TOPK]
    vals = np.take_along_axis(logits, idx, axis=1)
    vals = vals - vals.max(axis=1, keepdims=True)
    ev = np.exp(vals)
    probs = (ev / ev.sum(axis=1, keepdims=True)).astype(np.float32)
    return idx.astype(np.int64), probs


def kernel(x, router_w, w1, b1, w2, b2):
    global last_results

    x = np.asarray(x, dtype=np.float32)
    router_w = np.asarray(router_w, dtype=np.float32)
    w1 = np.asarray(w1, dtype=np.float32)
    b1 = np.asarray(b1, dtype=np.float32)
    w2 = np.asarray(w2, dtype=np.float32)
    b2 = np.asarray(b2, dtype=np.float32)

    orig_shape = x.shape
    xf = x.reshape(-1, D)

    idx, probs = _route(xf, router_w)

    # Group the (token, k) pairs by expert; gpos = column in the sorted order.
    flat_e = idx.ravel()  # entry j corresponds to token j//2, slot j%2
    order = np.argsort(flat_e, kind="stable")
    counts = np.bincount(flat_e, minlength=E)
    starts = np.zeros(E + 1, dtype=np.int64)
    np.cumsum(counts, out=starts[1:])
    rank = np.empty(TP, dtype=np.int64)
    rank[order] = np.arange(TP, dtype=np.int64) - starts[flat_e[order]]
    gpos = (rank + starts[flat_e]).reshape(T, TOPK)

    nc = _get_program(_schedule(counts))

    xt = np.ascontiguousarray(xf.astype(bfloat16)[order // 2].T)  # [D, TP]
    in_maps = []
    for c in range(E):
        fs = slice(c * FS, (c + 1) * FS)
        w1c = np.ascontiguousarray(
            w1[:, fs, :].transpose(0, 2, 1)
        ).astype(bfloat16)                                        # [E, D, FS]
        w2c = np.ascontiguousarray(
            w2[:, :, fs].transpose(0, 2, 1)
        ).astype(bfloat16)                                        # [E, FS, D]
        b1c = np.ascontiguousarray(
            b1[:, fs].reshape(E, MF, P).transpose(2, 0, 1)
        )                                                         # [P, E, MF]
        in_maps.append({"xt": xt, "w1s": w1c, "w2s": w2c, "b1s": b1c})

    res = run_bass_kernel_spmd(nc, in_maps, core_ids=list(range(E)), **trace_kwargs)
    last_results = res

    acc = np.zeros((D, TP), dtype=np.float32)
    for r in res.results:
        acc += np.asarray(r["yt"]).astype(np.float32)
    accT = acc.T                                                  # [TP, D]
    out = probs[:, 0:1] * (accT[gpos[:, 0]] + b2[idx[:, 0]])
    out += probs[:, 1:2] * (accT[gpos[:, 1]] + b2[idx[:, 1]])
    return out.astype(np.float32).reshape(orig_shape)
